# revision 1
# baseline (speedup 1.0000x reference)
"""Trainium2 Bass kernel for DeepSupervisionLoss (focal + boundary-weighted dice,
4 heads, deep supervision). Pure data-parallel over the batch dim across 8 cores;
each core reduces its shard to 16 partial scalars; host combines.

Math (per element, t binary, z = x*(2t-1)):
  bce  = softplus(-z) = ln(1 + exp(-z))
  pt=v = sigmoid(z)   = exp(-bce)
  u^2.5 = exp(-2.5*(z + bce))           (u = 1 - pt)
  focal_elem = 0.25 * u^2.5 * bce
  boundary b = maxpool3x3(t) + maxpool3x3(1-t) - 1   (in {0,1})
  w = 1 + 7b;  wt = w*t;  w0 = w*(1-t)
  I  = <v, wt>;  P = I + (S_w0 - <v, w0>);  T = S_wt
  dice = (2I+1)/(P+T+1);  head = 0.3*focal_mean + 0.7*(1-dice)
Sums <.,.> are computed on the PE as diagonals of accumulated A^T B in PSUM.
"""
import sys

import numpy as np

for _p in ("/opt/trn_rl_repo",):
    if _p not in sys.path:
        sys.path.insert(0, _p)

import ml_dtypes  # noqa: E402

import concourse.bacc as bacc  # noqa: E402
import concourse.mybir as mybir  # noqa: E402
from concourse import tile  # noqa: E402
from concourse.alu_op_type import AluOpType  # noqa: E402

F32 = mybir.dt.float32
BF16 = mybir.dt.bfloat16
AF = mybir.ActivationFunctionType

N_CORES = 8
N_IMG_TOTAL = 32
H = W = 512
P = 128              # partitions
RB = 4               # rows per partition
FD = RB * W          # 2048 free elems per image tile
NCH = 16             # 128-col chunks per tile
PRED_NAMES = ("main_pred", "ds1", "ds2", "ds3")


def build_nc(n_img, stage=4):
    # stage: 1=DMA only, 2=+target pipeline, 3=+pred ACT chain, 4=full (PE dots)
    nc = bacc.Bacc("TRN2", target_bir_lowering=False, debug=False)

    xs = [nc.declare_dram_parameter(nm, [n_img, H, W], F32, isOutput=False)
          for nm in PRED_NAMES]
    tg = nc.declare_dram_parameter("target", [n_img, H, W], F32, isOutput=False)
    wup_d = nc.declare_dram_parameter("wup", [P, P], BF16, isOutput=False)
    wdn_d = nc.declare_dram_parameter("wdn", [P, P], BF16, isOutput=False)
    ident_d = nc.declare_dram_parameter("ident", [P, P], F32, isOutput=False)
    ones_d = nc.declare_dram_parameter("onescol", [P, 1], F32, isOutput=False)
    out_d = nc.declare_dram_parameter("out", [1, 16], F32, isOutput=True)

    def img_view(dram, i):
        # [512, 512] image -> [128, 2048]; partition p holds rows 4p..4p+3
        return dram.ap()[i].rearrange("(p a) w -> p (a w)", p=P)

    with tile.TileContext(nc) as tc:
        with (
            tc.tile_pool(name="consts", bufs=1) as cp,
            tc.tile_pool(name="tgt", bufs=1) as tp_,
            tc.tile_pool(name="tgt2", bufs=2) as tp2,
            tc.tile_pool(name="pred", bufs=2) as pp,
            tc.tile_pool(name="xin", bufs=3) as xp,
            tc.tile_pool(name="psacc", bufs=1, space="PSUM") as pa,
            tc.tile_pool(name="pssh", bufs=1, space="PSUM") as ps,
        ):
            wup = cp.tile([P, P], BF16)
            wdn = cp.tile([P, P], BF16)
            ident = cp.tile([P, P], F32)
            onescol = cp.tile([P, 1], F32)
            nc.sync.dma_start(out=wup[:], in_=wup_d.ap())
            nc.sync.dma_start(out=wdn[:], in_=wdn_d.ap())
            nc.sync.dma_start(out=ident[:], in_=ident_d.ap())
            nc.sync.dma_start(out=onescol[:], in_=ones_d.ap())

            swt_cols = cp.tile([P, n_img], F32)
            sw0_cols = cp.tile([P, n_img], F32)
            acc16 = cp.tile([P, 16], F32)
            nc.vector.memset(acc16[:], 0.0)

            # PSUM accumulators: diag(A^T B) accumulation targets.
            # Pre-zeroed; all matmuls accumulate (start=False) so Tile's
            # PE reordering cannot race a start=True clear against earlier
            # contributions (order of pure accumulates is commutative).
            accIV = pa.tile([P, 8, P], F32)   # per pred: [I | vw0] (2 banks)
            accF = pa.tile([P, 4, P], F32)    # per pred: focal     (1 bank)
            nc.vector.memset(accIV[:], 0.0)
            nc.vector.memset(accF[:], 0.0)

            # padded horizontal buffers (pad cols stay zero forever)
            Px = cp.tile([P, RB, W + 4], BF16)
            Pn = cp.tile([P, RB, W + 4], BF16)
            nc.vector.memset(Px[:], 0.0)
            nc.vector.memset(Pn[:], 0.0)

            for img in range(n_img):
                # ---------------- target pipeline ----------------
                t_f32 = tp2.tile([P, FD], F32, name="t_f32")
                nc.sync.dma_start(out=t_f32[:], in_=img_view(tg, img))

                tb = tp_.tile([P, RB, W], BF16, name="tb")        # t  in bf16
                tp = tp_.tile([P, RB, W], BF16, name="tp")        # 1-t in bf16
                nc.vector.tensor_copy(out=tb[:], in_=t_f32[:].rearrange("p (a w) -> p a w", a=RB))
                nc.vector.tensor_scalar(
                    out=tp[:], in0=t_f32[:].rearrange("p (a w) -> p a w", a=RB),
                    scalar1=-1.0, scalar2=1.0, op0=AluOpType.mult, op1=AluOpType.add)

                msign = tp_.tile([P, FD], F32, name="msign")      # 2t-1 in f32
                nc.vector.tensor_scalar(
                    out=msign[:], in0=tb[:].rearrange("p a w -> p (a w)"),
                    scalar1=2.0, scalar2=-1.0, op0=AluOpType.mult, op1=AluOpType.add)

                # center copies into padded buffers (center at col offset 2)
                nc.vector.tensor_copy(out=Px[:, :, 2:W + 2], in_=tb[:])
                nc.vector.tensor_copy(out=Pn[:, :, 2:W + 2], in_=tp[:])

                # horizontal 3-tap max (zero pad is exact for binary data)
                Ax = tp_.tile([P, RB, W], BF16, name="Ax")
                hx = tp_.tile([P, RB, W], BF16, name="hx")
                nc.vector.tensor_tensor(out=Ax[:], in0=Px[:, :, 1:W + 1],
                                        in1=Px[:, :, 3:W + 3], op=AluOpType.max)
                nc.vector.tensor_tensor(out=hx[:], in0=Ax[:], in1=tb[:], op=AluOpType.max)
                An = tp_.tile([P, RB, W], BF16, name="An")
                hn = tp_.tile([P, RB, W], BF16, name="hn")
                nc.vector.tensor_tensor(out=An[:], in0=Pn[:, :, 1:W + 1],
                                        in1=Pn[:, :, 3:W + 3], op=AluOpType.max)
                nc.vector.tensor_tensor(out=hn[:], in0=An[:], in1=tp[:], op=AluOpType.max)

                # cross-partition rows via PE shift matrices (zero-fill rows,
                # exact for max of non-negative data)
                shx3 = ps.tile([P, W], F32, name="shx3")
                shx0 = ps.tile([P, W], F32, name="shx0")
                shn3 = ps.tile([P, W], F32, name="shn3")
                shn0 = ps.tile([P, W], F32, name="shn0")
                nc.tensor.matmul(shx3[:], wup[:], hx[:, 3, :], start=True, stop=True)
                nc.tensor.matmul(shx0[:], wdn[:], hx[:, 0, :], start=True, stop=True)
                nc.tensor.matmul(shn3[:], wup[:], hn[:, 3, :], start=True, stop=True)
                nc.tensor.matmul(shn0[:], wdn[:], hn[:, 0, :], start=True, stop=True)

                # vertical 3-tap max within/across partitions
                Dx = tp_.tile([P, RB, W], BF16, name="Dx")
                Dn = tp_.tile([P, RB, W], BF16, name="Dn")
                for (hsrc, dst, sh3, sh0) in ((hx, Dx, shx3, shx0), (hn, Dn, shn3, shn0)):
                    m12 = tp_.tile([P, W], BF16, name="m12")
                    nc.vector.tensor_tensor(out=m12[:], in0=hsrc[:, 1, :],
                                            in1=hsrc[:, 2, :], op=AluOpType.max)
                    nc.vector.tensor_tensor(out=dst[:, 1, :], in0=hsrc[:, 0, :],
                                            in1=m12[:], op=AluOpType.max)
                    nc.vector.tensor_tensor(out=dst[:, 2, :], in0=m12[:],
                                            in1=hsrc[:, 3, :], op=AluOpType.max)
                    v0a = tp_.tile([P, W], BF16, name="v0a")
                    nc.vector.tensor_tensor(out=v0a[:], in0=hsrc[:, 0, :],
                                            in1=hsrc[:, 1, :], op=AluOpType.max)
                    nc.vector.tensor_tensor(out=dst[:, 0, :], in0=v0a[:],
                                            in1=sh3[:], op=AluOpType.max)
                    v3a = tp_.tile([P, W], BF16, name="v3a")
                    nc.vector.tensor_tensor(out=v3a[:], in0=hsrc[:, 2, :],
                                            in1=hsrc[:, 3, :], op=AluOpType.max)
                    nc.vector.tensor_tensor(out=dst[:, 3, :], in0=v3a[:],
                                            in1=sh0[:], op=AluOpType.max)

                # b = Dx + Dn - 1 ;  W = 1 + 7b ;  wt = W*t ; w0 = W*(1-t)
                bb = tp_.tile([P, FD], BF16, name="bb")
                nc.vector.scalar_tensor_tensor(
                    out=bb[:], in0=Dx[:].rearrange("p a w -> p (a w)"), scalar=-1.0,
                    in1=Dn[:].rearrange("p a w -> p (a w)"),
                    op0=AluOpType.add, op1=AluOpType.add)
                Wt_ = tp_.tile([P, FD], BF16, name="Wt_")
                nc.vector.tensor_scalar(
                    out=Wt_[:], in0=bb[:], scalar1=7.0, scalar2=1.0,
                    op0=AluOpType.mult, op1=AluOpType.add)
                WW = tp_.tile([P, 2, FD], BF16, name="WW")
                nc.vector.scalar_tensor_tensor(
                    out=WW[:, 0, :], in0=Wt_[:], scalar=1.0,
                    in1=tb[:].rearrange("p a w -> p (a w)"),
                    op0=AluOpType.mult, op1=AluOpType.mult,
                    accum_out=swt_cols[:, img:img + 1])
                nc.vector.scalar_tensor_tensor(
                    out=WW[:, 1, :], in0=Wt_[:], scalar=1.0,
                    in1=tp[:].rearrange("p a w -> p (a w)"),
                    op0=AluOpType.mult, op1=AluOpType.mult,
                    accum_out=sw0_cols[:, img:img + 1])

                # ---------------- pred pipeline (4 heads) ----------------
                for k in range(4):
                    x_t = xp.tile([P, FD], F32, name="x_t")
                    # SWDGE: HWDGE descriptors have too few wait slots for
                    # this load's WAR deps (DVE reader + two DMA queues).
                    nc.gpsimd.dma_start(out=x_t[:], in_=img_view(xs[k], img))

                    z_t = pp.tile([P, FD], BF16, name="z_t")
                    nc.vector.tensor_tensor(out=z_t[:], in0=x_t[:], in1=msign[:],
                                            op=AluOpType.mult)
                    e_t = pp.tile([P, FD], F32, name="e_t")
                    nc.scalar.activation(e_t[:], z_t[:], AF.Exp, scale=-1.0)
                    bce_t = pp.tile([P, FD], BF16, name="bce_t")
                    nc.scalar.activation(bce_t[:], e_t[:], AF.Ln, bias=1.0)
                    v_t = pp.tile([P, FD], BF16, name="v_t")
                    nc.scalar.activation(v_t[:], bce_t[:], AF.Exp, scale=-1.0)
                    q_t = pp.tile([P, FD], BF16, name="q_t")
                    nc.vector.tensor_tensor(out=q_t[:], in0=z_t[:], in1=bce_t[:],
                                            op=AluOpType.add)
                    u25_t = pp.tile([P, FD], BF16, name="u25_t")
                    nc.scalar.activation(u25_t[:], q_t[:], AF.Exp, scale=-2.5)

                    last = img == n_img - 1
                    for c in range(NCH):
                        cs = slice(c * P, (c + 1) * P)
                        nc.tensor.matmul(
                            accIV[:, 2 * k:2 * k + 2, :],
                            v_t[:, cs], WW[:, :, cs],
                            start=False, stop=(last and c == NCH - 1),
                            skip_group_check=True)
                        nc.tensor.matmul(
                            accF[:, k, :],
                            u25_t[:, cs], bce_t[:, cs],
                            start=False, stop=(last and c == NCH - 1),
                            skip_group_check=True)

            # ---------------- final reduction ----------------
            nc.vector.tensor_reduce(out=acc16[:, 12:13], in_=swt_cols[:],
                                    axis=mybir.AxisListType.X, op=AluOpType.add)
            nc.vector.tensor_reduce(out=acc16[:, 13:14], in_=sw0_cols[:],
                                    axis=mybir.AxisListType.X, op=AluOpType.add)
            dscr = cp.tile([P, P], F32)
            for j in range(8):
                nc.vector.scalar_tensor_tensor(
                    out=dscr[:], in0=accIV[:, j, :], scalar=1.0, in1=ident[:],
                    op0=AluOpType.mult, op1=AluOpType.mult,
                    accum_out=acc16[:, j:j + 1])
            for j in range(4):
                nc.vector.scalar_tensor_tensor(
                    out=dscr[:], in0=accF[:, j, :], scalar=1.0, in1=ident[:],
                    op0=AluOpType.mult, op1=AluOpType.mult,
                    accum_out=acc16[:, 8 + j:9 + j])

            fin = ps.tile([1, 16], F32, name="fin")
            nc.tensor.matmul(fin[:], onescol[:], acc16[:], start=True, stop=True)
            out_sb = cp.tile([1, 16], F32)
            nc.vector.tensor_copy(out=out_sb[:], in_=fin[:])
            nc.sync.dma_start(out=out_d.ap(), in_=out_sb[:])

    _pin_act_table_set(nc)
    nc.finalize()
    return nc


def _pin_act_table_set(nc, set_name="natural_log_exp_and_others"):
    """All our ACT funcs (Exp, Ln) live in one table set, but the stock
    insertion pass alternates exp_and_others/natural_log per instruction
    (~31 reloads x ~1.3us on the critical ScalarE). Pin every load to the
    combined set and drop duplicates."""
    orig = nc.insert_act_table_loads

    def patched():
        orig()
        from concourse.hw_specs import get_activation_tables
        names = list(get_activation_tables(nc.m.arch).keys())
        cid = names.index(set_name)
        for fn in nc.m.functions:
            for blk in fn.blocks:
                seen = False
                kept = []
                for ins in blk.instructions:
                    if isinstance(ins, mybir.InstLoadActFuncSet):
                        if seen:
                            continue
                        ins.act_func_set_id = cid
                        seen = True
                    kept.append(ins)
                if len(kept) != len(blk.instructions):
                    blk.instructions[:] = kept

    nc.insert_act_table_loads = patched


def _consts():
    wup = np.eye(P, k=1).astype(ml_dtypes.bfloat16)   # out[p] = in[p-1], 0 at p=0
    wdn = np.eye(P, k=-1).astype(ml_dtypes.bfloat16)  # out[p] = in[p+1], 0 at p=127
    ident = np.eye(P, dtype=np.float32)
    ones = np.ones((P, 1), dtype=np.float32)
    return {"wup": wup, "wdn": wdn, "ident": ident, "onescol": ones}


_NC_CACHE = {}


def _get_nc(n_img):
    if n_img not in _NC_CACHE:
        _NC_CACHE[n_img] = build_nc(n_img)
    return _NC_CACHE[n_img]


def combine_partials(outs, n_total_elems):
    """outs: list of [1,16] f32 per core -> final scalar (float64 host math)."""
    s = np.zeros(16, dtype=np.float64)
    for o in outs:
        s += np.asarray(o, dtype=np.float64).reshape(16)
    I = [s[2 * k] for k in range(4)]
    VW0 = [s[2 * k + 1] for k in range(4)]
    F = [s[8 + k] for k in range(4)]
    S_wt, S_w0 = s[12], s[13]
    total = 0.0
    for k, c in enumerate((1.0, 0.4, 0.2, 0.1)):
        f = 0.25 * F[k] / n_total_elems
        Pk = I[k] + (S_w0 - VW0[k])
        dice = (2.0 * I[k] + 1.0) / (Pk + S_wt + 1.0)
        total += c * (0.3 * f + 0.7 * (1.0 - dice))
    return np.float32(total)


def kernel(main_pred, ds1, ds2, ds3, target, _trace=False):
    from concourse.bass_utils import run_bass_kernel_spmd

    n_img = N_IMG_TOTAL // N_CORES
    nc = _get_nc(n_img)
    consts = _consts()
    preds = {"main_pred": main_pred, "ds1": ds1, "ds2": ds2, "ds3": ds3}
    in_maps = []
    for core in range(N_CORES):
        sl = slice(core * n_img, (core + 1) * n_img)
        m = {nm: np.ascontiguousarray(
                np.asarray(v).reshape(N_IMG_TOTAL, H, W)[sl]).astype(np.float32)
             for nm, v in preds.items()}
        m["target"] = np.ascontiguousarray(
            np.asarray(target).reshape(N_IMG_TOTAL, H, W)[sl]).astype(np.float32)
        m.update(consts)
        in_maps.append(m)

    res = run_bass_kernel_spmd(nc, in_maps, list(range(N_CORES)), trace=_trace)
    outs = [r["out"] for r in res.results]
    total = combine_partials(outs, N_IMG_TOTAL * H * W)
    if _trace:
        kernel._last_result = res
    return np.asarray(total, dtype=np.float32)

